# revision 11
# baseline (speedup 1.0000x reference)
"""NonLocalBlock2D on 8 Trainium2 NeuronCores.

Data-parallel over batch: each core processes one batch element.
Per-core algorithm (x_b: [256, 4096] with N = H*W = 4096, C_INT = 128):
  theta = theta_w @ x + theta_b        [128, N]   (standard layout)
  phi   = phi_w @ x + phi_b            [128, N]
  gT    = (g_w @ x)^T                  [N, 128]   (computed directly transposed,
                                                   g bias folded into final bias)
  For each key tile j (128 keys):
    scoresT_j = phi_j^T-contract       [128 keys, N_q]  (keys on partitions)
    expT_j    = exp(scoresT_j)          (no max subtraction: |scores| < 33)
    yT       += gT_j^T @ expT_j         [128 c, N_q]  (PSUM accumulation)
    sum      += ones^T @ expT_j         [128, N_q]    (softmax denominator,
                                                       broadcast to 128 rows)
  out = (mask_w @ yT) * (1/sum) + (mask_w @ g_b + mask_b) + x

All matmuls run as float32r (full-rate fp32 streaming on the PE array,
~12-bit mantissa; bit layout == fp32 with low mantissa bits zero).  Every
matmul input is produced on-chip by a compute op with float32r output (the
required rounding step); DMA'd tiles are never fed to the PE directly --
that also keeps the per-matmul sync-wait count within walrus's LDW budget.
"""

import numpy as np

import concourse.bass as bass
import concourse.bacc as bacc
import concourse.mybir as mybir
import concourse.tile as tile
from concourse.bass import ds, ts
from concourse.bass_utils import run_bass_kernel_spmd

F32 = mybir.dt.float32
F32R = mybir.dt.float32r
EXP = mybir.ActivationFunctionType.Exp
ADD = mybir.AluOpType.add

C_IN = 256
C_INT = 128
N = 4096          # H*W
QT = 1024         # query tile (columns of scoresT)
NQT = N // QT     # 4
NKT = N // 128    # 32 key tiles


def build_nc():
    nc = bacc.Bacc()

    x = nc.declare_dram_parameter("x", [C_IN, N], F32, isOutput=False)
    thT = nc.declare_dram_parameter("thT", [C_IN, C_INT], F32, isOutput=False)
    phT = nc.declare_dram_parameter("phT", [C_IN, C_INT], F32, isOutput=False)
    gwT = nc.declare_dram_parameter("gwT", [C_IN, C_INT], F32, isOutput=False)
    mwT = nc.declare_dram_parameter("mwT", [C_INT, C_IN], F32, isOutput=False)
    th_b = nc.declare_dram_parameter("th_b", [C_INT, 1], F32, isOutput=False)
    ph_b = nc.declare_dram_parameter("ph_b", [C_INT, 1], F32, isOutput=False)
    fb = nc.declare_dram_parameter("fb", [C_IN, 1], F32, isOutput=False)
    out = nc.declare_dram_parameter("out", [C_IN, N], F32, isOutput=True)

    with tile.TileContext(nc) as tc:
        with (
            tc.tile_pool(name="singles", bufs=1) as singles,
            tc.tile_pool(name="big", bufs=1) as big,
            tc.tile_pool(name="stage", bufs=1) as stage,
            tc.tile_pool(name="expp", bufs=3) as expp,
            tc.tile_pool(name="sbw", bufs=2) as sbw,
            tc.tile_pool(name="ps_a", bufs=2, space="PSUM") as ps_a,
            tc.tile_pool(name="ps_y", bufs=1, space="PSUM") as ps_y,
            tc.tile_pool(name="ps_s", bufs=1, space="PSUM") as ps_s,
        ):
            # ---- load x and weights; round to fp32r via DVE copies ----
            # every load gets its own staging tile: slot reuse would put
            # WAR sem-waits on the DMACopy, and walrus's DMA descriptor
            # only has room for 2.
            x_sb = []
            for i in range(2):
                st = stage.tile([128, N], F32, tag=f"xs{i}")
                nc.gpsimd.dma_start(out=st, in_=x[ts(i, 128), :])
                t = big.tile([128, N], F32R, tag=f"x{i}")
                nc.vector.tensor_copy(out=t, in_=st)
                x_sb.append(t)

            def load_w(dram_slice, tag):
                st = stage.tile([128, C_INT], F32, tag=f"ws_{tag}")
                nc.gpsimd.dma_start(out=st, in_=dram_slice)
                t = singles.tile([128, C_INT], F32R, tag=tag)
                nc.vector.tensor_copy(out=t, in_=st)
                return t

            thT_sb = [load_w(thT[ts(i, 128), :], f"thT{i}") for i in range(2)]
            phT_sb = [load_w(phT[ts(i, 128), :], f"phT{i}") for i in range(2)]
            gwT_sb = [load_w(gwT[ts(i, 128), :], f"gwT{i}") for i in range(2)]
            mwT_sb = [load_w(mwT[:, ts(i, 128)], f"mwT{i}") for i in range(2)]
            def load_vec(dram_slice, tag):
                st = stage.tile([128, 1], F32, tag=f"vs_{tag}")
                nc.gpsimd.dma_start(out=st, in_=dram_slice)
                t = singles.tile([128, 1], F32, tag=tag)
                nc.vector.tensor_copy(out=t, in_=st)
                return t

            th_b_sb = load_vec(th_b[:, :], "th_b")
            ph_b_sb = load_vec(ph_b[:, :], "ph_b")
            fb_sb = [load_vec(fb[ts(i, 128), :], f"fb{i}") for i in range(2)]
            ones_st = stage.tile([128, 128], F32, tag="ones_st")
            nc.vector.memset(ones_st, 1.0)
            ones_sb = singles.tile([128, 128], F32R, tag="ones")
            nc.vector.tensor_copy(out=ones_sb, in_=ones_st)

            # ---- projections ----
            th_sb = big.tile([128, N], F32R, tag="th")
            ph_sb = big.tile([128, N], F32R, tag="ph")
            gT_sb = big.tile([128, N], F32R, tag="gT")

            for w_sb, b_sb, dst in (
                (thT_sb, th_b_sb, th_sb),
                (phT_sb, ph_b_sb, ph_sb),
            ):
                for nb in range(N // 512):
                    ps = ps_a.tile([128, 512], F32, tag="pa")
                    for k in range(2):
                        nc.tensor.matmul(
                            ps,
                            w_sb[k],
                            x_sb[k][:, ts(nb, 512)],
                            start=(k == 0),
                            stop=(k == 1),
                        )
                    nc.vector.tensor_scalar_add(
                        out=dst[:, ts(nb, 512)], in0=ps, scalar1=b_sb
                    )

            # gT: [keys (partitions), c] per 128-key chunk j at columns j*128
            for j in range(NKT):
                ps = ps_a.tile([128, 128], F32, tag="pa")
                for k in range(2):
                    nc.tensor.matmul(
                        ps,
                        x_sb[k][:, ts(j, 128)],
                        gwT_sb[k],
                        start=(k == 0),
                        stop=(k == 1),
                    )
                nc.vector.tensor_copy(out=gT_sb[:, ts(j, 128)], in_=ps)

            # ---- attention + mask conv, per query tile ----
            for qt in range(NQT):
                yT_ps = ps_y.tile([128, QT], F32, tag="y")
                sm_ps = ps_s.tile([128, QT], F32, tag="s")
                for j in range(NKT):
                    sc_ps = ps_a.tile([128, QT], F32, tag="pa")
                    for h in range(QT // 512):
                        nc.tensor.matmul(
                            sc_ps[:, ts(h, 512)],
                            ph_sb[:, ts(j, 128)],
                            th_sb[:, ds(qt * QT + h * 512, 512)],
                            start=True,
                            stop=True,
                        )
                    ex = expp.tile([128, QT], F32R, tag="ex")
                    nc.scalar.activation(out=ex, in_=sc_ps, func=EXP)
                    for h in range(QT // 512):
                        nc.tensor.matmul(
                            yT_ps[:, ts(h, 512)],
                            gT_sb[:, ts(j, 128)],
                            ex[:, ts(h, 512)],
                            start=(j == 0),
                            stop=(j == NKT - 1),
                        )
                        nc.tensor.matmul(
                            sm_ps[:, ts(h, 512)],
                            ones_sb,
                            ex[:, ts(h, 512)],
                            start=(j == 0),
                            stop=(j == NKT - 1),
                        )

                recip = sbw.tile([128, QT], F32, tag="rc")
                nc.vector.reciprocal(recip, sm_ps)
                yT_sb = sbw.tile([128, QT], F32R, tag="ysb")
                nc.scalar.copy(out=yT_sb, in_=yT_ps)

                for c in range(2):
                    for h in range(QT // 512):
                        mk = ps_a.tile([128, 512], F32, tag="pa")
                        nc.tensor.matmul(
                            mk,
                            mwT_sb[c],
                            yT_sb[:, ts(h, 512)],
                            start=True,
                            stop=True,
                        )
                        t = sbw.tile([128, 512], F32, tag="ot")
                        nc.vector.tensor_mul(t, mk, recip[:, ts(h, 512)])
                        o = sbw.tile([128, 512], F32, tag="o2")
                        nc.vector.scalar_tensor_tensor(
                            out=o,
                            in0=t,
                            scalar=fb_sb[c],
                            in1=x_sb[c][:, ds(qt * QT + h * 512, 512)].bitcast(F32),
                            op0=ADD,
                            op1=ADD,
                        )
                        nc.gpsimd.dma_start(
                            out=out[ds(c * 128, 128), ds(qt * QT + h * 512, 512)],
                            in_=o,
                        )
    return nc


def _prep_inputs(inputs):
    f = lambda k: np.ascontiguousarray(np.asarray(inputs[k], dtype=np.float32))
    x = f("x")
    g_w, g_b = f("g_w"), f("g_b")
    theta_w, theta_b = f("theta_w"), f("theta_b")
    phi_w, phi_b = f("phi_w"), f("phi_b")
    mask_w, mask_b = f("mask_w"), f("mask_b")
    base = {
        "thT": np.ascontiguousarray(theta_w.T),
        "phT": np.ascontiguousarray(phi_w.T),
        "gwT": np.ascontiguousarray(g_w.T),
        "mwT": np.ascontiguousarray(mask_w.T),
        "th_b": theta_b.reshape(C_INT, 1),
        "ph_b": phi_b.reshape(C_INT, 1),
        "fb": (mask_w @ g_b + mask_b).reshape(C_IN, 1),
    }
    B = x.shape[0]
    in_maps = [dict(base, x=x[b].reshape(C_IN, N)) for b in range(B)]
    return in_maps


def kernel(**inputs):
    in_maps = _prep_inputs(inputs)
    nc = build_nc()
    nc.finalize()
    res = run_bass_kernel_spmd(nc, in_maps, list(range(len(in_maps))))
    return np.stack(
        [r["out"].reshape(C_IN, 64, 64) for r in res.results], axis=0
    )


def run_traced(inputs):
    """Like kernel(), but also returns the BassKernelResults (with profile)."""
    in_maps = _prep_inputs(inputs)
    nc = build_nc()
    nc.finalize()
    res = run_bass_kernel_spmd(nc, in_maps, list(range(len(in_maps))), trace=True)
    out = np.stack(
        [r["out"].reshape(C_IN, 64, 64) for r in res.results], axis=0
    )
    return out, res


# revision 13
# speedup vs baseline: 1.2176x; 1.2176x over previous
"""NonLocalBlock2D on 8 Trainium2 NeuronCores.

Data-parallel over batch: each core processes one batch element.
Per-core algorithm (x_b: [256, 4096] with N = H*W = 4096, C_INT = 128):
  theta = theta_w @ x + theta_b        [128, N]   (standard layout)
  phi   = phi_w @ x + phi_b            [128, N]
  gT    = (g_w @ x)^T                  [N, 128]   (computed directly transposed,
                                                   g bias folded into final bias)
  For each key tile j (128 keys):
    scoresT_j = phi_j^T-contract       [128 keys, N_q]  (keys on partitions)
    expT_j    = exp(scoresT_j)          (no max subtraction: |scores| < 33)
    yT       += gT_j^T @ expT_j         [128 c, N_q]  (PSUM accumulation)
    sum      += ones^T @ expT_j         [128, N_q]    (softmax denominator,
                                                       broadcast to 128 rows)
  out = (mask_w @ yT) * (1/sum) + (mask_w @ g_b + mask_b) + x

All matmuls run as float32r (full-rate fp32 streaming on the PE array,
~12-bit mantissa; bit layout == fp32 with low mantissa bits zero).  Every
matmul input is produced on-chip by a compute op with float32r output (the
required rounding step); DMA'd tiles are never fed to the PE directly --
that also keeps the per-matmul sync-wait count within walrus's LDW budget.
"""

import numpy as np

import concourse.bass as bass
import concourse.bacc as bacc
import concourse.mybir as mybir
import concourse.tile as tile
from concourse.bass import ds, ts
from concourse.bass_utils import run_bass_kernel_spmd

F32 = mybir.dt.float32
F32R = mybir.dt.float32r
EXP = mybir.ActivationFunctionType.Exp
ADD = mybir.AluOpType.add

C_IN = 256
C_INT = 128
N = 4096          # H*W
QT = 1024         # query tile (columns of scoresT)
NQT = N // QT     # 4
NKT = N // 128    # 32 key tiles


def build_nc():
    nc = bacc.Bacc()

    x = nc.declare_dram_parameter("x", [C_IN, N], F32, isOutput=False)
    thT = nc.declare_dram_parameter("thT", [C_IN, C_INT], F32, isOutput=False)
    phT = nc.declare_dram_parameter("phT", [C_IN, C_INT], F32, isOutput=False)
    gwT = nc.declare_dram_parameter("gwT", [C_IN, C_INT], F32, isOutput=False)
    mwT = nc.declare_dram_parameter("mwT", [C_INT, C_IN], F32, isOutput=False)
    th_b = nc.declare_dram_parameter("th_b", [C_INT, 1], F32, isOutput=False)
    ph_b = nc.declare_dram_parameter("ph_b", [C_INT, 1], F32, isOutput=False)
    fb = nc.declare_dram_parameter("fb", [C_IN, 1], F32, isOutput=False)
    out = nc.declare_dram_parameter("out", [C_IN, N], F32, isOutput=True)

    with tile.TileContext(nc) as tc:
        with (
            tc.tile_pool(name="singles", bufs=1) as singles,
            tc.tile_pool(name="big", bufs=1) as big,
            tc.tile_pool(name="stage", bufs=1) as stage,
            tc.tile_pool(name="expp", bufs=4) as expp,
            tc.tile_pool(name="sbw", bufs=2) as sbw,
            tc.tile_pool(name="ps_a", bufs=2, space="PSUM") as ps_a,
            tc.tile_pool(name="ps_y", bufs=1, space="PSUM") as ps_y,
            tc.tile_pool(name="ps_s", bufs=1, space="PSUM") as ps_s,
        ):
            # ---- load x and weights; round to fp32r via DVE copies ----
            # every load gets its own staging tile: slot reuse would put
            # WAR sem-waits on the DMACopy, and walrus's DMA descriptor
            # only has room for 2.
            x_sb = []
            for i in range(2):
                st = stage.tile([128, N], F32, tag=f"xs{i}")
                nc.gpsimd.dma_start(out=st, in_=x[ts(i, 128), :])
                t = big.tile([128, N], F32R, tag=f"x{i}")
                nc.vector.tensor_copy(out=t, in_=st)
                x_sb.append(t)

            def load_w(dram_slice, tag):
                st = stage.tile([128, C_INT], F32, tag=f"ws_{tag}")
                nc.gpsimd.dma_start(out=st, in_=dram_slice)
                t = singles.tile([128, C_INT], F32R, tag=tag)
                nc.vector.tensor_copy(out=t, in_=st)
                return t

            thT_sb = [load_w(thT[ts(i, 128), :], f"thT{i}") for i in range(2)]
            phT_sb = [load_w(phT[ts(i, 128), :], f"phT{i}") for i in range(2)]
            gwT_sb = [load_w(gwT[ts(i, 128), :], f"gwT{i}") for i in range(2)]
            mwT_sb = [load_w(mwT[:, ts(i, 128)], f"mwT{i}") for i in range(2)]
            def load_vec(dram_slice, tag):
                st = stage.tile([128, 1], F32, tag=f"vs_{tag}")
                nc.gpsimd.dma_start(out=st, in_=dram_slice)
                t = singles.tile([128, 1], F32, tag=tag)
                nc.vector.tensor_copy(out=t, in_=st)
                return t

            th_b_sb = load_vec(th_b[:, :], "th_b")
            ph_b_sb = load_vec(ph_b[:, :], "ph_b")
            fb_sb = [load_vec(fb[ts(i, 128), :], f"fb{i}") for i in range(2)]
            ones_st = stage.tile([128, 128], F32, tag="ones_st")
            nc.vector.memset(ones_st, 1.0)
            ones_sb = singles.tile([128, 128], F32R, tag="ones")
            nc.vector.tensor_copy(out=ones_sb, in_=ones_st)

            # ---- projections ----
            th_sb = big.tile([128, N], F32R, tag="th")
            ph_sb = big.tile([128, N], F32R, tag="ph")
            gT_sb = big.tile([128, N], F32R, tag="gT")

            for w_sb, b_sb, dst in (
                (thT_sb, th_b_sb, th_sb),
                (phT_sb, ph_b_sb, ph_sb),
            ):
                for nb in range(N // 512):
                    ps = ps_a.tile([128, 512], F32, tag="pa")
                    for k in range(2):
                        nc.tensor.matmul(
                            ps,
                            w_sb[k],
                            x_sb[k][:, ts(nb, 512)],
                            start=(k == 0),
                            stop=(k == 1),
                        )
                    nc.vector.tensor_scalar_add(
                        out=dst[:, ts(nb, 512)], in0=ps, scalar1=b_sb
                    )

            # gT: [keys (partitions), c] per 128-key chunk j at columns j*128
            for j in range(NKT):
                ps = ps_a.tile([128, 128], F32, tag="pa")
                for k in range(2):
                    nc.tensor.matmul(
                        ps,
                        x_sb[k][:, ts(j, 128)],
                        gwT_sb[k],
                        start=(k == 0),
                        stop=(k == 1),
                    )
                nc.vector.tensor_copy(out=gT_sb[:, ts(j, 128)], in_=ps)

            # ---- attention + mask conv, per query tile ----
            # software-pipelined: scores for j+1 are emitted before the
            # yT matmuls for j, so the PE streams scores_{j+1} while the
            # scalar engine is still exp-ing scores_j.  The softmax
            # denominator is accumulated on the (otherwise idle) DVE as
            # 128 per-key-row partials, then collapsed across partitions
            # with two cheap ones-matmuls per query tile.
            def emit_scores(qt, j):
                sc_ps = ps_a.tile([128, QT], F32, tag="pa")
                for h in range(QT // 512):
                    nc.tensor.matmul(
                        sc_ps[:, ts(h, 512)],
                        ph_sb[:, ts(j, 128)],
                        th_sb[:, ds(qt * QT + h * 512, 512)],
                        start=True,
                        stop=True,
                    )
                return sc_ps

            for qt in range(NQT):
                yT_ps = ps_y.tile([128, QT], F32, tag="y")
                sc_ps = emit_scores(qt, 0)
                acc = None
                for j in range(NKT):
                    ex = expp.tile([128, QT], F32R, tag="ex")
                    nc.scalar.activation(out=ex, in_=sc_ps, func=EXP)
                    if j + 1 < NKT:
                        sc_ps = emit_scores(qt, j + 1)
                    for h in range(QT // 512):
                        nc.tensor.matmul(
                            yT_ps[:, ts(h, 512)],
                            gT_sb[:, ts(j, 128)],
                            ex[:, ts(h, 512)],
                            start=(j == 0),
                            stop=(j == NKT - 1),
                        )
                    if acc is None:
                        acc = ex
                    else:
                        dt_out = F32R if j == NKT - 1 else F32
                        nacc = expp.tile([128, QT], dt_out, tag="sacc")
                        nc.vector.tensor_tensor(
                            out=nacc,
                            in0=ex.bitcast(F32),
                            in1=acc.bitcast(F32),
                            op=ADD,
                        )
                        acc = nacc

                sm_ps = ps_s.tile([128, QT], F32, tag="s")
                for h in range(QT // 512):
                    nc.tensor.matmul(
                        sm_ps[:, ts(h, 512)],
                        ones_sb,
                        acc[:, ts(h, 512)],
                        start=True,
                        stop=True,
                    )
                recip = sbw.tile([128, QT], F32, tag="rc")
                nc.vector.reciprocal(recip, sm_ps)
                yT_sb = sbw.tile([128, QT], F32R, tag="ysb")
                nc.scalar.copy(out=yT_sb, in_=yT_ps)

                for c in range(2):
                    for h in range(QT // 512):
                        mk = ps_a.tile([128, 512], F32, tag="pa")
                        nc.tensor.matmul(
                            mk,
                            mwT_sb[c],
                            yT_sb[:, ts(h, 512)],
                            start=True,
                            stop=True,
                        )
                        t = sbw.tile([128, 512], F32, tag="ot")
                        nc.vector.tensor_mul(t, mk, recip[:, ts(h, 512)])
                        o = sbw.tile([128, 512], F32, tag="o2")
                        nc.vector.scalar_tensor_tensor(
                            out=o,
                            in0=t,
                            scalar=fb_sb[c],
                            in1=x_sb[c][:, ds(qt * QT + h * 512, 512)].bitcast(F32),
                            op0=ADD,
                            op1=ADD,
                        )
                        nc.gpsimd.dma_start(
                            out=out[ds(c * 128, 128), ds(qt * QT + h * 512, 512)],
                            in_=o,
                        )
    return nc


def _prep_inputs(inputs):
    f = lambda k: np.ascontiguousarray(np.asarray(inputs[k], dtype=np.float32))
    x = f("x")
    g_w, g_b = f("g_w"), f("g_b")
    theta_w, theta_b = f("theta_w"), f("theta_b")
    phi_w, phi_b = f("phi_w"), f("phi_b")
    mask_w, mask_b = f("mask_w"), f("mask_b")
    base = {
        "thT": np.ascontiguousarray(theta_w.T),
        "phT": np.ascontiguousarray(phi_w.T),
        "gwT": np.ascontiguousarray(g_w.T),
        "mwT": np.ascontiguousarray(mask_w.T),
        "th_b": theta_b.reshape(C_INT, 1),
        "ph_b": phi_b.reshape(C_INT, 1),
        "fb": (mask_w @ g_b + mask_b).reshape(C_IN, 1),
    }
    B = x.shape[0]
    in_maps = [dict(base, x=x[b].reshape(C_IN, N)) for b in range(B)]
    return in_maps


def kernel(**inputs):
    in_maps = _prep_inputs(inputs)
    nc = build_nc()
    nc.finalize()
    res = run_bass_kernel_spmd(nc, in_maps, list(range(len(in_maps))))
    return np.stack(
        [r["out"].reshape(C_IN, 64, 64) for r in res.results], axis=0
    )


def run_traced(inputs):
    """Like kernel(), but also returns the BassKernelResults (with profile)."""
    in_maps = _prep_inputs(inputs)
    nc = build_nc()
    nc.finalize()
    res = run_bass_kernel_spmd(nc, in_maps, list(range(len(in_maps))), trace=True)
    out = np.stack(
        [r["out"].reshape(C_IN, 64, 64) for r in res.results], axis=0
    )
    return out, res
